# revision 7
# baseline (speedup 1.0000x reference)
"""LIF spiking-neuron recurrence kernel for Trainium2 (8 NeuronCores, SPMD).

Problem: x [32, 100, 8192] f32, decay [1] f32.
    d = sigmoid(decay)
    mem_0 = x[:,0];  mem_t = mem_{t-1} * d * (1 - spike_{t-1}) + x[:,t]
    spike_t = (mem_t > 0.5);  out[:,t] = spike_t  (f32 0/1)

Device formulation (bit-exact vs the reference):
    W_{-1} = 0
    M_t = (W_{t-1} * d) + x_t              # one DVE scalar_tensor_tensor
    W_t = (M_t <= 0.5) * M_t               # one DVE scalar_tensor_tensor
spike_t = (M_t > 0.5) = (W_t == 0) exactly (W_t = M_t != 0 when no spike,
= +0.0 when spike), so the device only streams W back and the host emits
(W == 0). This makes the whole step exactly 2 DVE instructions.

Sharding: the 32*8192 = 262144 independent (b, d) lanes are split 8 ways by
feature blocks (d-shard): core c owns d in [1024c, 1024c+1024). Per-core
layout is [128 partitions, T*256] with partition p = b*4 + (d_local//256),
free offset = t*256 + d_local%256, so each timestep is a [128, 256] slice
and DMA lines are long and contiguous. No cross-core communication
(forward only).

Built with bacc.Bacc + compile() so multi-sem waits get legalized into
event-semaphore ladders (TRN2 allows at most 1 sem wait per instruction).
"""

from contextlib import ExitStack

import numpy as np

N_CORES = 8
B, T, D = 32, 100, 8192
P = 128          # SBUF partitions
F = 256          # free elements per timestep per core (32*1024/128)
TC = 20          # timesteps per DMA chunk
NCHUNK = T // TC
THRESH = 0.5

_BUILD_CACHE: dict = {}


def _build_nc(t_steps: int, tc: int, d_imm: float):
    import concourse.bass as bass
    import concourse.tile as tile
    from concourse import bacc, mybir

    nchunk = t_steps // tc
    assert nchunk * tc == t_steps

    nc = bacc.Bacc("TRN2", debug=False, target_bir_lowering=False)
    x_in = nc.dram_tensor("x", [P, t_steps * F], mybir.dt.float32,
                          kind="ExternalInput")
    w_out = nc.dram_tensor("w", [P, t_steps * F], mybir.dt.float32,
                           kind="ExternalOutput")

    mult = mybir.AluOpType.mult
    add = mybir.AluOpType.add
    is_le = mybir.AluOpType.is_le

    with tile.TileContext(nc) as tcx, ExitStack() as ctx:
        xpool = ctx.enter_context(tcx.tile_pool(name="xp", bufs=3))
        opool = ctx.enter_context(tcx.tile_pool(name="op", bufs=3))
        spool = ctx.enter_context(tcx.tile_pool(name="sp", bufs=1))

        zeros = spool.tile([P, F], mybir.dt.float32)
        nc.vector.memset(zeros[:], 0.0)
        m = spool.tile([P, F], mybir.dt.float32)

        w_prev = zeros[:]
        for c in range(nchunk):
            xt = xpool.tile([P, tc * F], mybir.dt.float32)
            nc.sync.dma_start(out=xt[:], in_=x_in[:, c * tc * F:(c + 1) * tc * F])
            ot = opool.tile([P, tc * F], mybir.dt.float32)
            for i in range(tc):
                xs = xt[:, i * F:(i + 1) * F]
                ws = ot[:, i * F:(i + 1) * F]
                # M = (W_prev * d) + x_t
                nc.vector.scalar_tensor_tensor(
                    out=m[:], in0=w_prev, scalar=d_imm, in1=xs,
                    op0=mult, op1=add)
                # W = (M <= 0.5) * M
                nc.vector.scalar_tensor_tensor(
                    out=ws, in0=m[:], scalar=THRESH, in1=m[:],
                    op0=is_le, op1=mult)
                w_prev = ws
            nc.sync.dma_start(out=w_out[:, c * tc * F:(c + 1) * tc * F],
                              in_=ot[:])
    nc.compile()
    return nc


def _get_nc(t_steps: int, tc: int, d_imm: float):
    key = (t_steps, tc, np.float32(d_imm).tobytes())
    if key not in _BUILD_CACHE:
        _BUILD_CACHE[key] = _build_nc(t_steps, tc, d_imm)
    return _BUILD_CACHE[key]


def _shard_x(x: np.ndarray) -> list[np.ndarray]:
    b, t, d = x.shape
    # [b, t, core, chunk, 256] -> [core, b, chunk, t, 256] -> [core, 128, t*256]
    xr = x.reshape(b, t, N_CORES, 4, F).transpose(2, 0, 3, 1, 4)
    xr = np.ascontiguousarray(xr).reshape(N_CORES, P, t * F)
    return [xr[c] for c in range(N_CORES)]


def _unshard_w(w8: np.ndarray, t: int) -> np.ndarray:
    # [core, 128, t*256] -> [core, b, chunk, t, 256] -> [b, t, core, chunk, 256]
    wr = w8.reshape(N_CORES, B, 4, t, F).transpose(1, 3, 0, 2, 4)
    return np.ascontiguousarray(wr).reshape(B, t, N_CORES * 4 * F)


def _sigmoid_f32(decay: np.ndarray) -> np.float32:
    import jax
    import jax.numpy as jnp
    d = np.asarray(jax.nn.sigmoid(jnp.asarray(decay, jnp.float32)))
    return np.float32(d.reshape(-1)[0])


def kernel(x: np.ndarray, decay: np.ndarray) -> np.ndarray:
    from concourse.bass_utils import run_bass_kernel_spmd

    x = np.asarray(x, dtype=np.float32)
    b, t, d = x.shape
    d_f32 = _sigmoid_f32(np.asarray(decay))

    nc = _get_nc(t, TC if t % TC == 0 else t, float(d_f32))
    shards = _shard_x(x)
    in_maps = [{"x": np.ascontiguousarray(s)} for s in shards]
    res = run_bass_kernel_spmd(nc, in_maps, core_ids=list(range(N_CORES)))
    w8 = np.stack([res.results[c]["w"] for c in range(N_CORES)], axis=0)
    w = _unshard_w(w8, t)
    return (w == 0.0).astype(np.float32)


# revision 8
# speedup vs baseline: 1.5924x; 1.5924x over previous
"""LIF spiking-neuron recurrence kernel for Trainium2 (8 NeuronCores, SPMD).

Problem: x [32, 100, 8192] f32, decay [1] f32.
    d = sigmoid(decay)
    mem_0 = x[:,0];  mem_t = mem_{t-1} * d * (1 - spike_{t-1}) + x[:,t]
    spike_t = (mem_t > 0.5);  out[:,t] = spike_t  (f32 0/1)

Device formulation (bit-exact vs the reference):
    W_{-1} = 0
    M_t = (W_{t-1} * d) + x_t
    W_t = (M_t <= 0.5) * M_t
spike_t = (M_t > 0.5) = (W_t == 0) exactly (W_t = M_t != 0 when no spike,
= +0.0 when spike), so the device streams W back (as bf16 — any nonzero f32
stays nonzero in bf16) and the host emits (W == 0).

The whole step is ONE custom DVE op (registered at runtime through the
concourse custom-DVE table mechanism):
    LIF_STEP_ANT: out = M * (M <= s1),  M = in0*s0 + in1
Each ALU stage rounds in f32 exactly like the reference's mult/add chain,
and the *(0/1) mask multiply is exact, so results match the reference
bit-for-bit.

Sharding: the 32*8192 = 262144 independent (b, d) lanes are split 8 ways by
feature blocks (d-shard): core c owns d in [1024c, 1024c+1024). Per-core
layout is [128 partitions, T*256] with partition p = b*4 + (d_local//256),
free offset = t*256 + d_local%256, so each timestep is a [128, 256] slice
and DMA lines are long and contiguous. No cross-core communication
(forward only).

Chunked DMA schedule: a small first chunk so compute starts early, bulk
20-step chunks (2.56 MB loads), small last chunk so the tail flush is
short. Input loads are HWDGE (nc.sync); output stores are SWDGE
(nc.gpsimd) because they cast f32->bf16 in flight.
"""

from contextlib import ExitStack

import numpy as np

N_CORES = 8
B, T, D = 32, 100, 8192
P = 128          # SBUF partitions
F = 256          # free elements per timestep per core (32*1024/128)
THRESH = 0.5
OUT_BF16 = True

_BUILD_CACHE: dict = {}
_LIF_OP = None


def _chunk_schedule(t_steps: int) -> list[int]:
    if t_steps == 100:
        return [2, 6, 12, 20, 20, 20, 12, 8]
    chunks = []
    rem = t_steps
    while rem > 0:
        c = min(20, rem)
        chunks.append(c)
        rem -= c
    return chunks


def _get_lif_op():
    """Register the fused LIF-step custom DVE op (idempotent)."""
    global _LIF_OP
    if _LIF_OP is not None:
        return _LIF_OP
    from concourse.dve_ops import (
        CUSTOM_DVE_SPECS, OPS, _SUB_OPCODE_FOR_NAME, DveOp,
    )
    from concourse.dve_spec import C0, C1, Spec, Src0, Src1, lower
    from concourse.dve_table_gen import dve_ver_for
    from concourse.dve_uop import DveOpSpec

    name = "LIF_STEP_ANT"
    if name in _SUB_OPCODE_FOR_NAME:
        _LIF_OP = next(op for op in OPS if op.name == name)
        return _LIF_OP

    M = Src0 * C0 + Src1

    def _ref(in0, in1, s0, s1, imm2):
        m = (in0.astype(np.float32) * np.float32(s0)
             + in1.astype(np.float32)).astype(np.float32)
        return np.where(m <= np.float32(s1), m, np.float32(0.0)).astype(np.float32)

    spec = Spec(body=M * (M <= C1), reference=_ref)
    row = max(_SUB_OPCODE_FOR_NAME.values()) + 1
    assert row < 0x20
    _SUB_OPCODE_FOR_NAME[name] = row
    shas = {}
    for ver in ("v3",):  # TRN2
        tmp = DveOpSpec(name=name, opcode=row, uops=lower(spec, ver=ver),
                        rd1_en=True)
        shas[ver] = tmp.sha(ver)
    assert dve_ver_for("TRN2") == "v3"
    op = DveOp(name, spec, subdim=False, uops_sha=shas)
    OPS.append(op)
    CUSTOM_DVE_SPECS[name] = spec
    _LIF_OP = op
    return op


def _build_nc(t_steps: int, d_imm: float):
    import concourse.tile as tile
    from concourse import bacc, mybir

    lif_op = _get_lif_op()
    chunks = _chunk_schedule(t_steps)
    assert sum(chunks) == t_steps
    max_tc = max(chunks)
    out_dt = mybir.dt.bfloat16 if OUT_BF16 else mybir.dt.float32

    nc = bacc.Bacc("TRN2", debug=False, target_bir_lowering=False)
    x_in = nc.dram_tensor("x", [P, t_steps * F], mybir.dt.float32,
                          kind="ExternalInput")
    w_out = nc.dram_tensor("w", [P, t_steps * F], out_dt,
                           kind="ExternalOutput")

    with tile.TileContext(nc) as tcx, ExitStack() as ctx:
        xpool = ctx.enter_context(tcx.tile_pool(name="xp", bufs=3))
        opool = ctx.enter_context(tcx.tile_pool(name="op", bufs=3))
        spool = ctx.enter_context(tcx.tile_pool(name="sp", bufs=1))

        zeros = spool.tile([P, F], mybir.dt.float32)
        nc.vector.memset(zeros[:], 0.0)

        w_prev = zeros[:]
        t0 = 0
        for tc in chunks:
            xt = xpool.tile([P, max_tc * F], mybir.dt.float32, tag="xt")
            nc.sync.dma_start(out=xt[:, :tc * F],
                              in_=x_in[:, t0 * F:(t0 + tc) * F])
            ot = opool.tile([P, max_tc * F], mybir.dt.float32, tag="ot")
            for i in range(tc):
                xs = xt[:, i * F:(i + 1) * F]
                ws = ot[:, i * F:(i + 1) * F]
                # W_t = M*(M<=0.5), M = W_{t-1}*d + x_t  — one DVE op
                nc.vector._custom_dve(lif_op, out=ws, in0=w_prev, in1=xs,
                                      s0=d_imm, s1=THRESH)
                w_prev = ws
            if OUT_BF16:
                nc.gpsimd.dma_start(out=w_out[:, t0 * F:(t0 + tc) * F],
                                    in_=ot[:, :tc * F])
            else:
                nc.sync.dma_start(out=w_out[:, t0 * F:(t0 + tc) * F],
                                  in_=ot[:, :tc * F])
            t0 += tc
    nc.compile()
    return nc


def _get_nc(t_steps: int, d_imm: float):
    key = (t_steps, np.float32(d_imm).tobytes())
    if key not in _BUILD_CACHE:
        _BUILD_CACHE[key] = _build_nc(t_steps, d_imm)
    return _BUILD_CACHE[key]


def _shard_x(x: np.ndarray) -> list[np.ndarray]:
    b, t, d = x.shape
    # [b, t, core, chunk, 256] -> [core, b, chunk, t, 256] -> [core, 128, t*256]
    xr = x.reshape(b, t, N_CORES, 4, F).transpose(2, 0, 3, 1, 4)
    xr = np.ascontiguousarray(xr).reshape(N_CORES, P, t * F)
    return [xr[c] for c in range(N_CORES)]


def _unshard_spikes(w8: np.ndarray, t: int) -> np.ndarray:
    # spike = (W == 0); [core, 128, t*256] -> [b, t, D]
    s = (w8 == 0).astype(np.float32)
    sr = s.reshape(N_CORES, B, 4, t, F).transpose(1, 3, 0, 2, 4)
    return np.ascontiguousarray(sr).reshape(B, t, N_CORES * 4 * F)


def _sigmoid_f32(decay: np.ndarray) -> np.float32:
    import jax
    import jax.numpy as jnp
    d = np.asarray(jax.nn.sigmoid(jnp.asarray(decay, jnp.float32)))
    return np.float32(d.reshape(-1)[0])


def kernel(x: np.ndarray, decay: np.ndarray) -> np.ndarray:
    from concourse.bass_utils import run_bass_kernel_spmd

    x = np.asarray(x, dtype=np.float32)
    b, t, d = x.shape
    d_f32 = _sigmoid_f32(np.asarray(decay))

    nc = _get_nc(t, float(d_f32))
    shards = _shard_x(x)
    in_maps = [{"x": np.ascontiguousarray(s)} for s in shards]
    res = run_bass_kernel_spmd(nc, in_maps, core_ids=list(range(N_CORES)))
    w8 = np.stack([np.asarray(res.results[c]["w"]) for c in range(N_CORES)],
                  axis=0)
    return _unshard_spikes(w8, t)
